# revision 33
# baseline (speedup 1.0000x reference)
"""Trainium2 Bass kernel for ButterflyGlobalLinear:

    y = x @ (mask * weight)^T + bias

x: [16384, 2048] f32, weight/mask: [2048, 2048] f32, bias: [2048] f32.

The mask is a banded butterfly (|out - in| <= ~101) plus a dense first row
(output 0 reads all inputs) and dense first column (input 0 feeds all
outputs).  At 128-block granularity W^T is block-tridiagonal (46 blocks)
plus the global row/col, so the kernel only does ~1/5 of the dense work.

Design (measured 73us on 8 cores, rel err 3.2e-4 vs the 2e-2 gate):

  - data-parallel over tokens: 8 shards of 2048 tokens, one per NeuronCore
  - fp16 single-pass everywhere: same PE rate as bf16 (1 col/cycle) with
    8 extra mantissa bits; fp8/DoubleRow variants fail the gate or cost
    as much as fp16 once enough correction terms are added
  - per output block bo (128 outputs), contract only over input blocks
    {bo-1, bo, bo+1}; x^T and the packed W^T band live resident in SBUF
  - PSUM groups are 1024 cols (2 banks): prewrite/evac engine ops carry
    ~350ns fixed overhead each, so double-width ops halve that tax
  - the dense i=0 input row (a rank-1 update) plus the bias are fused
    into one per-group PSUM pre-write on the ACT engine:
    ps = Identity(x0_bcast * w_in0[bo] + bias[bo]); the band matmuls
    accumulate on top with start=False.  This removes 56 PE matmuls.
    Every ps bank gets a start=True zero matmul at program start: the
    first start=False accumulation on a virgin bank does not reliably
    include engine-prewritten content.
  - x0_bcast comes from a GPSIMD partition_broadcast of row 0 of the
    first x load (no extra DMA); evacuation (psum -> fp16 staging) runs
    on the Vector engine; PE/ACT/DVE all land ~45us busy, balanced
  - the dense o=0 output column is 14 single-output (M=1) matmuls per
    token-slab, packed 4-wide into the PE array via tile_position column
    groups; the 4 partial rows are stored and folded into y[:, 0] on the
    host (unsharding the K-parallel reduction)
  - y is staged fp16 and stored fp16 ([out, tok], transposed) halving
    store traffic; the host un-transposes and upcasts
  - every dma_start costs ~600ns of sequencer descriptor-gen time, so
    loads are BATCHED into ~10 triggers each for x (partition-major
    layout makes any block range one contiguous-row transfer) and W
  - stores ride the sync queue, issued after all loads: the per-queue
    FIFO gives loads absolute priority for HBM bandwidth, and store
    triggers on the idle sync engine never block the ACT pre-writes
  - group order: bo=1 first (cheap, needs no x0 broadcast), 15 down to
    3, then bo=0 (needs all x blocks for the o=0 column), ending on the
    cheap bo=2

BGL_MODE env selects the dtype experiment (default fp16):
  fp16   - single fp16 pass (default)
  bf16   - single bf16 pass (for comparison)
"""

import os

import numpy as np
import ml_dtypes

import concourse.bass as bass  # noqa: F401  (bass types via bacc)
import concourse.mybir as mybir
import concourse.tile as tile
from concourse import bacc
from concourse.bass_utils import run_bass_kernel_spmd


def _ensure_axon_hooks():
    """run_bass_kernel_spmd(trace=True) imports antenv.axon_hooks, which some
    images lack. Register the real libaxon-backed hook if available, else a
    no-op, so a BASS_TRACE=1 environment profiles instead of crashing."""
    import sys
    import types

    try:
        import antenv.axon_hooks  # noqa: F401
        return
    except ImportError:
        pass
    hook = None
    try:
        from trn_agent_boot.trn_boot import _ntff_profile_via_ctypes

        hook = _ntff_profile_via_ctypes("/opt/axon/libaxon_pjrt.so")
    except Exception:
        hook = None
    mod = types.ModuleType("antenv.axon_hooks")
    mod.get_axon_ntff_profile_hook = lambda: hook
    sys.modules["antenv.axon_hooks"] = mod


_ensure_axon_hooks()

MODE = os.environ.get("BGL_MODE", "fp16")

N_CORES = 8
TOK = 16384
F = 2048
P = 128
NB = F // P            # 16 feature blocks
NFREE = 512            # psum free dim (one bank of fp32)

F32 = mybir.dt.float32
BF16 = mybir.dt.bfloat16
FP16 = mybir.dt.float16

# most recent run's results (exec_time_ns etc.) for test harnesses
LAST_RESULTS = None


def _kset(bo):
    """Band input blocks contracted for output block bo (tridiagonal)."""
    return [bi for bi in (bo - 1, bo, bo + 1) if 0 <= bi < NB]


# Start at bo=1: the only cheap group that does not read the x0
# broadcast (its band carries the global row), so the PE can start while
# the GPSIMD broadcast of x row 0 (sourced from the first x load) still
# runs. Then walk 15 down to 3 (each adds one new x block), bo=0 (needs
# every x block for the o=0 column) second-to-last, and end on bo=2
# (nothing new to load, 3 matmuls) for a cheap tail.
BO_ORDER = [1] + list(range(15, 2, -1)) + [0, 2]


def _load_kset(bo):
    """x blocks whose tiles bo's group consumes (bo=0 also feeds the
    column-packed o=0 global reduction over every block)."""
    return list(range(NB)) if bo == 0 else _kset(bo)


def _wblocks():
    """(bo, bi) pairs needing a W^T block, in device compute order (so the
    packed slab can be streamed in exactly the order it is consumed)."""
    return [(bo, bi) for bo in BO_ORDER for bi in _kset(bo)]


_NC_CACHE = {}


def _build_nc(mode, tok_sh):
    """Build + compile the per-core Bass module (SPMD: same NEFF, 8 cores)."""
    if (mode, tok_sh) in _NC_CACHE:
        return _NC_CACHE[(mode, tok_sh)]
    wdt = {"fp16": FP16, "bf16": BF16}[mode]
    ns_count = tok_sh // NFREE
    blocks = _wblocks()
    bidx = {pair: i for i, pair in enumerate(blocks)}

    nc = bacc.Bacc("TRN2", target_bir_lowering=False, debug=False)

    # W^T blocks host-packed dense: column slab j holds block (bo,bi)=blocks[j]
    w_dram = nc.dram_tensor("w", [P, len(blocks) * P], wdt, kind="ExternalInput")
    # x^T partition-major: row p holds [block0 | block1 | ...] so any
    # column-range load is one DMA trigger with 4-32KB contiguous rows
    x_dram = nc.dram_tensor("x", [P, NB * tok_sh], wdt, kind="ExternalInput")
    # bias and the dense-input W^T row, merged into one load:
    # [:, :NB] = bias ([p, bo] -> bias[bo*128+p]); [:, NB:] = gwrow
    # ([p, bo] -> W^T[0, bo*128+p], cols 0,1 zeroed on the host since
    # blocks 0/1 carry the global row via the band)
    bg_dram = nc.dram_tensor("bg_pf", [P, 2 * NB], F32, kind="ExternalInput")
    # o=0 output column of W^T, blocked: column bi = W^T[bi*128:(bi+1)*128, 0]
    w0_dram = nc.dram_tensor("w0col", [P, NB], wdt, kind="ExternalInput")
    yt_dram = nc.dram_tensor("yt", [F, tok_sh], wdt, kind="ExternalOutput")
    # o=0 column partial sums (one row per tile_position column group);
    # the host folds these into y[:, 0] (unsharding the K-parallel split)
    psg_dram = nc.dram_tensor("psg", [4, tok_sh], F32, kind="ExternalOutput")

    with tile.TileContext(nc) as tc:
        with (
            tc.tile_pool(name="wpool", bufs=1) as wpool,
            tc.tile_pool(name="xpool", bufs=1) as xpool,
            tc.tile_pool(name="gpool", bufs=1) as gpool,
            tc.tile_pool(name="pspool", bufs=8, space="PSUM") as pspool,
            tc.tile_pool(name="opool", bufs=2) as opool,
        ):
            # the merged bias/gwrow scalars are the FIRST sync-ring trigger
            # (one 16KB dma): the first pre-write needs them ~8us in, and
            # the ACT ring would serialize them behind its table load
            x0b_sb = gpool.tile([P, tok_sh], wdt, tag="x0b")
            bg_sb = gpool.tile([P, 2 * NB], F32, tag="bg")
            nc.sync.dma_start(bg_sb[:], bg_dram[:, :])
            w0_sb = gpool.tile([P, NB], wdt, tag="w0col")
            nc.scalar.dma_start(w0_sb[:], w0_dram[:, :])

            # x^T fully resident in one [128, NB*tok_sh] tile; block bi
            # occupies columns [bi*tok_sh, (bi+1)*tok_sh)
            xall = xpool.tile([P, NB * tok_sh], wdt, tag="xall", name="xall")
            # resident packed W^T slab; block j = [:, j*128:(j+1)*128]
            wsb = wpool.tile([P, len(blocks) * P], wdt, tag="w", name="w")

            # Issue loads in first-use order, but BATCHED: every dma_start
            # costs ~600ns of sequencer descriptor-gen time, so 33 per-tile
            # triggers would take ~20us just to issue. Singles up front for
            # a fast PE start, then progressively larger merged ranges.
            # x blocks in first-use order: 14,15,13,12,...,2,1,0 (descending
            # after the first two), so merged ranges stay contiguous.
            xbatches = [[0], [1, 2], [14, 15], [13], [12], [11, 10], [9, 8],
                        [7, 6], [5, 4], [3]]
            # W-chunk batches (j ranges follow BO_ORDER, so ranges merge)
            wbatches = [[1], [15], [14], [13], [12, 11], [10, 9], [8, 7],
                        [6, 5], [4, 3], [0, 2]]
            wseq = []
            for bos in wbatches:
                js = [bidx[(bo, bi)] for bo in bos for bi in _kset(bo)]
                wseq.append((min(js), max(js) + 1))
            # interleave: w batch, then x batch, in consumption order
            for k in range(max(len(xbatches), len(wseq))):
                if k < len(wseq):
                    jlo, jhi = wseq[k]
                    nc.sync.dma_start(
                        wsb[:, jlo * P : jhi * P], w_dram[:, jlo * P : jhi * P]
                    )
                if k < len(xbatches):
                    bis = xbatches[k]
                    lo, hi = min(bis), max(bis) + 1
                    nc.sync.dma_start(
                        xall[:, lo * tok_sh : hi * tok_sh],
                        x_dram[:, lo * tok_sh : hi * tok_sh],
                    )
                if k == 0:
                    # broadcast x row 0 (= partition 0 of the first x load)
                    # across partitions for the global-input pre-writes; on
                    # the idle GPSIMD, done by ~9us, first needed by the
                    # second group
                    nc.gpsimd.partition_broadcast(
                        x0b_sb[:], xall[0:1, 0:tok_sh]
                    )

            nw = 2 * NFREE  # 1024-col groups: 2 psum banks, halves the
            nw_count = tok_sh // nw  # fixed per-op overhead of prewrite/evac

            # Prime every ps bank with a start=True zero matmul before the
            # real groups: the first start=False accumulation on a virgin
            # bank does not reliably include engine-prewritten content
            # (observed as garbage in exactly the first use of each bank).
            # Zero operands come from memsets, so priming runs during the
            # DMA ramp and costs nothing on the critical path.
            zcol = gpool.tile([1, P], wdt, tag="zcol", name="zcol")
            nc.gpsimd.memset(zcol[:], 0.0)
            zrow = gpool.tile([1, NFREE], wdt, tag="zrow", name="zrow")
            nc.gpsimd.memset(zrow[:], 0.0)
            for _ in range(3):
                pp = pspool.tile([P, nw], F32, tag="ps", bufs=3)
                for h in range(2):
                    nc.tensor.matmul(
                        pp[:, h * NFREE : (h + 1) * NFREE],
                        zcol[:],
                        zrow[:],
                        start=True,
                        stop=True,
                    )
            # (PE warm-up matmuls were tried here and measured neutral: the
            # DVFS step to full clock happens at a ~fixed time after kernel
            # start regardless of PE activity pattern.)

            for gi, bo in enumerate(BO_ORDER):
                ostage = opool.tile([P, tok_sh], wdt, tag="o", bufs=16)
                for nsw in range(nw_count):
                    wsl = slice(nsw * nw, (nsw + 1) * nw)
                    ps2 = pspool.tile([P, nw], F32, tag="ps", bufs=3)

                    # fused bias + dense-input rank-1 pre-write on the ACT
                    # engine: ps = Identity(x0b * w_in0[bo] + bias[bo]).
                    # gwrow cols 0,1 are 0 (bias only); those groups read
                    # already-loaded xall columns instead of x0b so the
                    # first group doesn't wait on the GPSIMD broadcast.
                    pw_in = (
                        x0b_sb[:, wsl]
                        if bo >= 2
                        else xall[:, 0 : nw]
                    )
                    nc.scalar.activation(
                        ps2[:],
                        pw_in,
                        mybir.ActivationFunctionType.Identity,
                        bias=bg_sb[:, bo : bo + 1],
                        scale=bg_sb[:, NB + bo : NB + bo + 1],
                    )

                    ks = _kset(bo)
                    for h in range(2):
                        tsl = slice((2 * nsw + h) * NFREE, (2 * nsw + h + 1) * NFREE)
                        for i, bi in enumerate(ks):
                            j = bidx[(bo, bi)]
                            nc.tensor.matmul(
                                ps2[:, h * NFREE : (h + 1) * NFREE],
                                wsb[:, j * P : (j + 1) * P],
                                xall[:, bi * tok_sh + tsl.start : bi * tok_sh + tsl.stop],
                                start=False,
                                stop=(i == len(ks) - 1),
                                skip_group_check=True,
                            )

                    # evacuate psum -> fp16 staging on DVE (the ACT engine is
                    # busy with the pre-writes); the final group's evac is
                    # split across ACT+DVE so the last store starts ~0.6us
                    # earlier (ACT is idle by then)
                    if gi == len(BO_ORDER) - 1:
                        nc.scalar.activation(
                            ostage[:, wsl.start : wsl.start + NFREE],
                            ps2[:, 0:NFREE],
                            mybir.ActivationFunctionType.Copy,
                        )
                        nc.vector.tensor_copy(
                            ostage[:, wsl.start + NFREE : wsl.stop],
                            ps2[:, NFREE : nw],
                        )
                        # store each 1024-half as soon as its evac lands so
                        # the final store overlaps the final evac
                        nc.sync.dma_start(
                            yt_dram[bo * P : (bo + 1) * P, wsl],
                            ostage[:, wsl],
                        )
                    else:
                        nc.vector.tensor_copy(ostage[:, wsl], ps2[:])

                if bo == 0:
                    # o=0 global column: every block bi>=2 contributes a
                    # single-output (M=1) matmul. Pack them 4-wide into the
                    # PE array via tile_position column groups so four
                    # stream concurrently; the 4 partial rows are stored and
                    # folded into y[:, 0] on the host (unsharding the
                    # K-parallel reduction).
                    psg_stage = gpool.tile(
                        [P, tok_sh], F32, tag="psg_stage", name="psg_stage"
                    )
                    units = list(range(2, NB))
                    ngrp = 4
                    per_grp = [[] for _ in range(ngrp)]
                    for idx, u in enumerate(units):
                        per_grp[idx % ngrp].append(u)
                    order = []
                    for slot in range(max(len(g) for g in per_grp)):
                        for jg in range(ngrp):
                            if slot < len(per_grp[jg]):
                                order.append((jg, slot, per_grp[jg][slot]))
                    for ns in range(ns_count):
                        tsl = slice(ns * NFREE, (ns + 1) * NFREE)
                        psg = pspool.tile(
                            [P, NFREE], F32, tag="psg", bufs=2, name="psg"
                        )
                        for jg, slot, bi in order:
                            nc.tensor.matmul(
                                psg[32 * jg : 32 * jg + 1, :],
                                w0_sb[:, bi : bi + 1],
                                xall[:, bi * tok_sh + tsl.start : bi * tok_sh + tsl.stop],
                                start=(slot == 0),
                                stop=(slot == len(per_grp[jg]) - 1),
                                tile_position=(0, 32 * jg),
                            )
                        # DMA cannot read PSUM: stage partitions 0..96 to
                        # SBUF in one DVE copy (cost is column-driven); the
                        # batched store happens after the slab loop
                        nc.vector.tensor_copy(
                            psg_stage[0:97, tsl], psg[0:97, :]
                        )

                if bo == 0:
                    for jg in range(4):
                        nc.sync.dma_start(
                            psg_dram[jg, :],
                            psg_stage[32 * jg : 32 * jg + 1, :],
                        )

                # stores ride the SYNC queue, issued after every load: the
                # per-queue FIFO then gives loads absolute priority over
                # stores for HBM bandwidth (no mid-run contention), and the
                # idle sync engine's store triggers never block the ACT
                # queue's pre-writes. ostage is buffered per-group (bufs=16)
                # so held-back stores don't stall evacuation.
                if gi != len(BO_ORDER) - 1:  # last group stored per-half above
                    nc.sync.dma_start(
                        yt_dram[bo * P : (bo + 1) * P, :], ostage[:]
                    )

    nc.compile()
    _NC_CACHE[(mode, tok_sh)] = nc
    return nc


def _prep_inputs(x, mask, weight, bias, mode, tok_sh):
    """Host-side layout prep -> per-core input maps."""
    npdt = {"fp16": np.float16, "bf16": ml_dtypes.bfloat16}[mode]
    n_sh = x.shape[0] // tok_sh

    w = mask.astype(np.float32) * weight.astype(np.float32)
    wtr = np.ascontiguousarray(w.T)  # [in, out]

    # pack the needed W^T blocks into a dense [128, nblocks*128] slab
    blocks = _wblocks()
    packed = np.empty((P, len(blocks) * P), dtype=np.float32)
    for j, (bo, bi) in enumerate(blocks):
        packed[:, j * P : (j + 1) * P] = wtr[
            bi * P : (bi + 1) * P, bo * P : (bo + 1) * P
        ]
    w_pk = packed.astype(npdt)

    # o=0 output column of W^T, blocked [128, NB]
    w0col = np.ascontiguousarray(wtr[:, 0].reshape(NB, P).T).astype(npdt)

    bias_pf = bias.astype(np.float32).reshape(NB, P).T

    # dense-input row of W^T, blocked [128, NB]; zero the columns whose
    # band blocks already carry the global row (input block 0 in bo=0,1)
    gwrow = wtr[0, :].astype(np.float32).reshape(NB, P).T.copy()
    gwrow[:, 0] = 0.0
    gwrow[:, 1] = 0.0
    bg_pf = np.ascontiguousarray(np.concatenate([bias_pf, gwrow], axis=1))

    # per-core transposed x shards, partition-major: [core, 128, NB*tok]
    xs = x.reshape(n_sh, tok_sh, F).transpose(0, 2, 1)
    x_h = np.ascontiguousarray(xs).astype(npdt).reshape(n_sh, NB, P, tok_sh)
    x_pm = np.ascontiguousarray(x_h.transpose(0, 2, 1, 3)).reshape(
        n_sh, P, NB * tok_sh
    )

    in_maps = []
    for c in range(n_sh):
        in_maps.append(
            {
                "bg_pf": bg_pf,
                "w": w_pk,
                "w0col": w0col,
                "x": x_pm[c],
            }
        )
    return in_maps


def kernel(x, mask, weight, bias):
    global LAST_RESULTS
    x = np.asarray(x)
    tok, f = x.shape
    assert (tok, f) == (TOK, F), (tok, f)
    tok_sh = tok // N_CORES

    nc = _build_nc(MODE, tok_sh)
    in_maps = _prep_inputs(
        np.asarray(x), np.asarray(mask), np.asarray(weight), np.asarray(bias),
        MODE, tok_sh,
    )
    res = run_bass_kernel_spmd(nc, in_maps, list(range(N_CORES)))
    LAST_RESULTS = res

    y = np.empty((tok, F), dtype=np.float32)
    for c in range(N_CORES):
        sl = slice(c * tok_sh, (c + 1) * tok_sh)
        y[sl, :] = res.results[c]["yt"].T.astype(np.float32)
        # unshard the K-parallel o=0 column reduction: fold the 4
        # column-group partial rows into y[:, 0]
        y[sl, 0] += res.results[c]["psg"].astype(np.float32).sum(axis=0)
    return y


# revision 35
# speedup vs baseline: 1.0046x; 1.0046x over previous
"""Trainium2 Bass kernel for ButterflyGlobalLinear:

    y = x @ (mask * weight)^T + bias

x: [16384, 2048] f32, weight/mask: [2048, 2048] f32, bias: [2048] f32.

The mask is a banded butterfly (|out - in| <= ~101) plus a dense first row
(output 0 reads all inputs) and dense first column (input 0 feeds all
outputs).  At 128-block granularity W^T is block-tridiagonal (46 blocks)
plus the global row/col, so the kernel only does ~1/5 of the dense work.

Design (measured 73us on 8 cores, rel err 3.2e-4 vs the 2e-2 gate):

  - data-parallel over tokens: 8 shards of 2048 tokens, one per NeuronCore
  - fp16 single-pass everywhere: same PE rate as bf16 (1 col/cycle) with
    8 extra mantissa bits; fp8/DoubleRow variants fail the gate or cost
    as much as fp16 once enough correction terms are added
  - per output block bo (128 outputs), contract only over input blocks
    {bo-1, bo, bo+1}; x^T and the packed W^T band live resident in SBUF
  - PSUM groups are 1024 cols (2 banks): prewrite/evac engine ops carry
    ~350ns fixed overhead each, so double-width ops halve that tax
  - the dense i=0 input row (a rank-1 update) plus the bias are fused
    into one per-group PSUM pre-write on the ACT engine:
    ps = Identity(x0_bcast * w_in0[bo] + bias[bo]); the band matmuls
    accumulate on top with start=False.  This removes 56 PE matmuls.
    Every ps bank gets a start=True zero matmul at program start: the
    first start=False accumulation on a virgin bank does not reliably
    include engine-prewritten content.
  - x0_bcast comes from a GPSIMD partition_broadcast of row 0 of the
    first x load (no extra DMA); evacuation (psum -> fp16 staging) runs
    on the Vector engine; PE/ACT/DVE all land ~45us busy, balanced
  - the dense o=0 output column is 14 single-output (M=1) matmuls per
    token-slab, packed 4-wide into the PE array via tile_position column
    groups; the 4 partial rows are stored and folded into y[:, 0] on the
    host (unsharding the K-parallel reduction)
  - y is staged fp16 and stored fp16 ([out, tok], transposed) halving
    store traffic; the host un-transposes and upcasts
  - every dma_start costs ~600ns of sequencer descriptor-gen time, so
    loads are BATCHED into ~10 triggers each for x (partition-major
    layout makes any block range one contiguous-row transfer) and W
  - stores ride the sync queue, issued after all loads: the per-queue
    FIFO gives loads absolute priority for HBM bandwidth, and store
    triggers on the idle sync engine never block the ACT pre-writes
  - group order: bo=1 first (cheap, needs no x0 broadcast), 15 down to
    3, then bo=0 (needs all x blocks for the o=0 column), ending on the
    cheap bo=2

BGL_MODE env selects the dtype experiment (default fp16):
  fp16   - single fp16 pass (default)
  bf16   - single bf16 pass (for comparison)
"""

import os

import numpy as np
import ml_dtypes

import concourse.bass as bass  # noqa: F401  (bass types via bacc)
import concourse.mybir as mybir
import concourse.tile as tile
from concourse import bacc
from concourse.bass_utils import run_bass_kernel_spmd


def _ensure_axon_hooks():
    """run_bass_kernel_spmd(trace=True) imports antenv.axon_hooks, which some
    images lack. Register the real libaxon-backed hook if available, else a
    no-op, so a BASS_TRACE=1 environment profiles instead of crashing."""
    import sys
    import types

    try:
        import antenv.axon_hooks  # noqa: F401
        return
    except ImportError:
        pass
    hook = None
    try:
        from trn_agent_boot.trn_boot import _ntff_profile_via_ctypes

        hook = _ntff_profile_via_ctypes("/opt/axon/libaxon_pjrt.so")
    except Exception:
        hook = None
    mod = types.ModuleType("antenv.axon_hooks")
    mod.get_axon_ntff_profile_hook = lambda: hook
    sys.modules["antenv.axon_hooks"] = mod


_ensure_axon_hooks()

MODE = os.environ.get("BGL_MODE", "fp16")

N_CORES = 8
TOK = 16384
F = 2048
P = 128
NB = F // P            # 16 feature blocks
NFREE = 512            # psum free dim (one bank of fp32)

F32 = mybir.dt.float32
BF16 = mybir.dt.bfloat16
FP16 = mybir.dt.float16

# most recent run's results (exec_time_ns etc.) for test harnesses
LAST_RESULTS = None


def _kset(bo):
    """Band input blocks contracted for output block bo (tridiagonal)."""
    return [bi for bi in (bo - 1, bo, bo + 1) if 0 <= bi < NB]


# Start at bo=1: the only cheap group that does not read the x0
# broadcast (its band carries the global row), so the PE can start while
# the GPSIMD broadcast of x row 0 (sourced from the first x load) still
# runs. Then walk 15 down to 3 (each adds one new x block), bo=0 (needs
# every x block for the o=0 column) second-to-last, and end on bo=2
# (nothing new to load, 3 matmuls) for a cheap tail.
BO_ORDER = [1] + list(range(15, 2, -1)) + [0, 2]


def _load_kset(bo):
    """x blocks whose tiles bo's group consumes (bo=0 also feeds the
    column-packed o=0 global reduction over every block)."""
    return list(range(NB)) if bo == 0 else _kset(bo)


def _wblocks():
    """(bo, bi) pairs needing a W^T block, in device compute order (so the
    packed slab can be streamed in exactly the order it is consumed)."""
    return [(bo, bi) for bo in BO_ORDER for bi in _kset(bo)]


_NC_CACHE = {}


def _build_nc(mode, tok_sh):
    """Build + compile the per-core Bass module (SPMD: same NEFF, 8 cores)."""
    if (mode, tok_sh) in _NC_CACHE:
        return _NC_CACHE[(mode, tok_sh)]
    wdt = {"fp16": FP16, "bf16": BF16}[mode]
    ns_count = tok_sh // NFREE
    blocks = _wblocks()
    bidx = {pair: i for i, pair in enumerate(blocks)}

    nc = bacc.Bacc("TRN2", target_bir_lowering=False, debug=False)

    # W^T blocks host-packed dense: column slab j holds block (bo,bi)=blocks[j]
    w_dram = nc.dram_tensor("w", [P, len(blocks) * P], wdt, kind="ExternalInput")
    # x^T partition-major: row p holds [block0 | block1 | ...] so any
    # column-range load is one DMA trigger with 4-32KB contiguous rows
    x_dram = nc.dram_tensor("x", [P, NB * tok_sh], wdt, kind="ExternalInput")
    # bias and the dense-input W^T row, merged into one load:
    # [:, :NB] = bias ([p, bo] -> bias[bo*128+p]); [:, NB:] = gwrow
    # ([p, bo] -> W^T[0, bo*128+p], cols 0,1 zeroed on the host since
    # blocks 0/1 carry the global row via the band)
    bg_dram = nc.dram_tensor("bg_pf", [P, 2 * NB], F32, kind="ExternalInput")
    # o=0 output column of W^T, blocked: column bi = W^T[bi*128:(bi+1)*128, 0]
    w0_dram = nc.dram_tensor("w0col", [P, NB], wdt, kind="ExternalInput")
    yt_dram = nc.dram_tensor("yt", [F, tok_sh], wdt, kind="ExternalOutput")
    # o=0 column partial sums (one row per tile_position column group);
    # the host folds these into y[:, 0] (unsharding the K-parallel split)
    psg_dram = nc.dram_tensor("psg", [4, tok_sh], F32, kind="ExternalOutput")

    with tile.TileContext(nc) as tc:
        with (
            tc.tile_pool(name="wpool", bufs=1) as wpool,
            tc.tile_pool(name="xpool", bufs=1) as xpool,
            tc.tile_pool(name="gpool", bufs=1) as gpool,
            tc.tile_pool(name="pspool", bufs=8, space="PSUM") as pspool,
            tc.tile_pool(name="opool", bufs=2) as opool,
        ):
            # the merged bias/gwrow scalars are the FIRST sync-ring trigger
            # (one 16KB dma): the first pre-write needs them ~8us in, and
            # the ACT ring would serialize them behind its table load
            x0b_sb = gpool.tile([P, tok_sh], wdt, tag="x0b")
            bg_sb = gpool.tile([P, 2 * NB], F32, tag="bg")
            nc.sync.dma_start(bg_sb[:], bg_dram[:, :])
            w0_sb = gpool.tile([P, NB], wdt, tag="w0col")
            nc.scalar.dma_start(w0_sb[:], w0_dram[:, :])

            # x^T fully resident in one [128, NB*tok_sh] tile; block bi
            # occupies columns [bi*tok_sh, (bi+1)*tok_sh)
            xall = xpool.tile([P, NB * tok_sh], wdt, tag="xall", name="xall")
            # resident packed W^T slab; block j = [:, j*128:(j+1)*128]
            wsb = wpool.tile([P, len(blocks) * P], wdt, tag="w", name="w")

            # Issue loads in first-use order, but BATCHED: every dma_start
            # costs ~600ns of sequencer descriptor-gen time, so 33 per-tile
            # triggers would take ~20us just to issue. Singles up front for
            # a fast PE start, then progressively larger merged ranges.
            # x blocks in first-use order: 14,15,13,12,...,2,1,0 (descending
            # after the first two), so merged ranges stay contiguous.
            xbatches = [[0], [1, 2], [14, 15], [13], [12], [11, 10], [9, 8],
                        [7, 6], [5, 4], [3]]
            # W-chunk batches (j ranges follow BO_ORDER, so ranges merge)
            wbatches = [[1], [15], [14], [13], [12, 11], [10, 9], [8, 7],
                        [6, 5], [4, 3], [0, 2]]
            wseq = []
            for bos in wbatches:
                js = [bidx[(bo, bi)] for bo in bos for bi in _kset(bo)]
                wseq.append((min(js), max(js) + 1))
            # interleave: w batch, then x batch, in consumption order
            for k in range(max(len(xbatches), len(wseq))):
                if k < len(wseq):
                    jlo, jhi = wseq[k]
                    nc.sync.dma_start(
                        wsb[:, jlo * P : jhi * P], w_dram[:, jlo * P : jhi * P]
                    )
                if k < len(xbatches):
                    bis = xbatches[k]
                    lo, hi = min(bis), max(bis) + 1
                    nc.sync.dma_start(
                        xall[:, lo * tok_sh : hi * tok_sh],
                        x_dram[:, lo * tok_sh : hi * tok_sh],
                    )
                if k == 0:
                    # broadcast x row 0 (= partition 0 of the first x load)
                    # across partitions for the global-input pre-writes; on
                    # the idle GPSIMD, done by ~9us, first needed by the
                    # second group
                    nc.gpsimd.partition_broadcast(
                        x0b_sb[:], xall[0:1, 0:tok_sh]
                    )

            nw = 2 * NFREE  # 1024-col groups: 2 psum banks, halves the
            nw_count = tok_sh // nw  # fixed per-op overhead of prewrite/evac

            # Prime every ps bank with a start=True zero matmul before the
            # real groups: the first start=False accumulation on a virgin
            # bank does not reliably include engine-prewritten content
            # (observed as garbage in exactly the first use of each bank).
            # Zero operands come from memsets, so priming runs during the
            # DMA ramp and costs nothing on the critical path.
            zcol = gpool.tile([1, P], wdt, tag="zcol", name="zcol")
            nc.gpsimd.memset(zcol[:], 0.0)
            zrow = gpool.tile([1, NFREE], wdt, tag="zrow", name="zrow")
            nc.gpsimd.memset(zrow[:], 0.0)
            for _ in range(3):
                pp = pspool.tile([P, nw], F32, tag="ps", bufs=3)
                for h in range(2):
                    nc.tensor.matmul(
                        pp[:, h * NFREE : (h + 1) * NFREE],
                        zcol[:],
                        zrow[:],
                        start=True,
                        stop=True,
                    )
            # (PE warm-up matmuls were tried here and measured neutral: the
            # DVFS step to full clock happens at a ~fixed time after kernel
            # start regardless of PE activity pattern.)

            for gi, bo in enumerate(BO_ORDER):
                ostage = opool.tile([P, tok_sh], wdt, tag="o", bufs=16)
                for nsw in range(nw_count):
                    wsl = slice(nsw * nw, (nsw + 1) * nw)
                    ps2 = pspool.tile([P, nw], F32, tag="ps", bufs=3)

                    # fused bias + dense-input rank-1 pre-write on the ACT
                    # engine: ps = Identity(x0b * w_in0[bo] + bias[bo]).
                    # gwrow cols 0,1 are 0 (bias only); those groups read
                    # already-loaded xall columns instead of x0b so the
                    # first group doesn't wait on the GPSIMD broadcast.
                    pw_in = (
                        x0b_sb[:, wsl]
                        if bo >= 2
                        else xall[:, 0 : nw]
                    )
                    nc.scalar.activation(
                        ps2[:],
                        pw_in,
                        mybir.ActivationFunctionType.Identity,
                        bias=bg_sb[:, bo : bo + 1],
                        scale=bg_sb[:, NB + bo : NB + bo + 1],
                    )

                    ks = _kset(bo)
                    for h in range(2):
                        tsl = slice((2 * nsw + h) * NFREE, (2 * nsw + h + 1) * NFREE)
                        for i, bi in enumerate(ks):
                            j = bidx[(bo, bi)]
                            nc.tensor.matmul(
                                ps2[:, h * NFREE : (h + 1) * NFREE],
                                wsb[:, j * P : (j + 1) * P],
                                xall[:, bi * tok_sh + tsl.start : bi * tok_sh + tsl.stop],
                                start=False,
                                stop=(i == len(ks) - 1),
                                skip_group_check=True,
                            )

                    # evacuate psum -> fp16 staging on DVE (the ACT engine is
                    # busy with the pre-writes); the final group's evac is
                    # split across ACT+DVE so the last store starts ~0.6us
                    # earlier (ACT is idle by then)
                    if gi == len(BO_ORDER) - 1:
                        nc.scalar.activation(
                            ostage[:, wsl.start : wsl.start + NFREE],
                            ps2[:, 0:NFREE],
                            mybir.ActivationFunctionType.Copy,
                        )
                        nc.vector.tensor_copy(
                            ostage[:, wsl.start + NFREE : wsl.stop],
                            ps2[:, NFREE : nw],
                        )
                    else:
                        nc.vector.tensor_copy(ostage[:, wsl], ps2[:])

                if bo == 0:
                    # o=0 global column: every block bi>=2 contributes a
                    # single-output (M=1) matmul. Pack them 4-wide into the
                    # PE array via tile_position column groups so four
                    # stream concurrently; the 4 partial rows are stored and
                    # folded into y[:, 0] on the host (unsharding the
                    # K-parallel reduction).
                    psg_stage = gpool.tile(
                        [P, tok_sh], F32, tag="psg_stage", name="psg_stage"
                    )
                    units = list(range(2, NB))
                    ngrp = 4
                    per_grp = [[] for _ in range(ngrp)]
                    for idx, u in enumerate(units):
                        per_grp[idx % ngrp].append(u)
                    order = []
                    for slot in range(max(len(g) for g in per_grp)):
                        for jg in range(ngrp):
                            if slot < len(per_grp[jg]):
                                order.append((jg, slot, per_grp[jg][slot]))
                    for ns in range(ns_count):
                        tsl = slice(ns * NFREE, (ns + 1) * NFREE)
                        psg = pspool.tile(
                            [P, NFREE], F32, tag="psg", bufs=2, name="psg"
                        )
                        for jg, slot, bi in order:
                            nc.tensor.matmul(
                                psg[32 * jg : 32 * jg + 1, :],
                                w0_sb[:, bi : bi + 1],
                                xall[:, bi * tok_sh + tsl.start : bi * tok_sh + tsl.stop],
                                start=(slot == 0),
                                stop=(slot == len(per_grp[jg]) - 1),
                                tile_position=(0, 32 * jg),
                            )
                        # DMA cannot read PSUM: stage partitions 0..96 to
                        # SBUF in one DVE copy (cost is column-driven); the
                        # batched store happens after the slab loop
                        nc.vector.tensor_copy(
                            psg_stage[0:97, tsl], psg[0:97, :]
                        )

                if bo == 0:
                    for jg in range(4):
                        nc.sync.dma_start(
                            psg_dram[jg, :],
                            psg_stage[32 * jg : 32 * jg + 1, :],
                        )

                # stores ride the SYNC queue, issued after every load: the
                # per-queue FIFO then gives loads absolute priority over
                # stores for HBM bandwidth (no mid-run contention), and the
                # idle sync engine's store triggers never block the ACT
                # queue's pre-writes. ostage is buffered per-group (bufs=16)
                # so held-back stores don't stall evacuation.
                if gi == len(BO_ORDER) - 1:
                    for nsw in range(nw_count):
                        wsl = slice(nsw * nw, (nsw + 1) * nw)
                        nc.sync.dma_start(
                            yt_dram[bo * P : (bo + 1) * P, wsl], ostage[:, wsl]
                        )
                else:
                    nc.sync.dma_start(
                        yt_dram[bo * P : (bo + 1) * P, :], ostage[:]
                    )

    nc.compile()
    _NC_CACHE[(mode, tok_sh)] = nc
    return nc


def _prep_inputs(x, mask, weight, bias, mode, tok_sh):
    """Host-side layout prep -> per-core input maps."""
    npdt = {"fp16": np.float16, "bf16": ml_dtypes.bfloat16}[mode]
    n_sh = x.shape[0] // tok_sh

    w = mask.astype(np.float32) * weight.astype(np.float32)
    wtr = np.ascontiguousarray(w.T)  # [in, out]

    # pack the needed W^T blocks into a dense [128, nblocks*128] slab
    blocks = _wblocks()
    packed = np.empty((P, len(blocks) * P), dtype=np.float32)
    for j, (bo, bi) in enumerate(blocks):
        packed[:, j * P : (j + 1) * P] = wtr[
            bi * P : (bi + 1) * P, bo * P : (bo + 1) * P
        ]
    w_pk = packed.astype(npdt)

    # o=0 output column of W^T, blocked [128, NB]
    w0col = np.ascontiguousarray(wtr[:, 0].reshape(NB, P).T).astype(npdt)

    bias_pf = bias.astype(np.float32).reshape(NB, P).T

    # dense-input row of W^T, blocked [128, NB]; zero the columns whose
    # band blocks already carry the global row (input block 0 in bo=0,1)
    gwrow = wtr[0, :].astype(np.float32).reshape(NB, P).T.copy()
    gwrow[:, 0] = 0.0
    gwrow[:, 1] = 0.0
    bg_pf = np.ascontiguousarray(np.concatenate([bias_pf, gwrow], axis=1))

    # per-core transposed x shards, partition-major: [core, 128, NB*tok]
    xs = x.reshape(n_sh, tok_sh, F).transpose(0, 2, 1)
    x_h = np.ascontiguousarray(xs).astype(npdt).reshape(n_sh, NB, P, tok_sh)
    x_pm = np.ascontiguousarray(x_h.transpose(0, 2, 1, 3)).reshape(
        n_sh, P, NB * tok_sh
    )

    in_maps = []
    for c in range(n_sh):
        in_maps.append(
            {
                "bg_pf": bg_pf,
                "w": w_pk,
                "w0col": w0col,
                "x": x_pm[c],
            }
        )
    return in_maps


def kernel(x, mask, weight, bias):
    global LAST_RESULTS
    x = np.asarray(x)
    tok, f = x.shape
    assert (tok, f) == (TOK, F), (tok, f)
    tok_sh = tok // N_CORES

    nc = _build_nc(MODE, tok_sh)
    in_maps = _prep_inputs(
        np.asarray(x), np.asarray(mask), np.asarray(weight), np.asarray(bias),
        MODE, tok_sh,
    )
    res = run_bass_kernel_spmd(nc, in_maps, list(range(N_CORES)))
    LAST_RESULTS = res

    y = np.empty((tok, F), dtype=np.float32)
    for c in range(N_CORES):
        sl = slice(c * tok_sh, (c + 1) * tok_sh)
        y[sl, :] = res.results[c]["yt"].T.astype(np.float32)
        # unshard the K-parallel o=0 column reduction: fold the 4
        # column-group partial rows into y[:, 0]
        y[sl, 0] += res.results[c]["psg"].astype(np.float32).sum(axis=0)
    return y


# revision 38
# speedup vs baseline: 1.0170x; 1.0124x over previous
"""Trainium2 Bass kernel for ButterflyGlobalLinear:

    y = x @ (mask * weight)^T + bias

x: [16384, 2048] f32, weight/mask: [2048, 2048] f32, bias: [2048] f32.

The mask is a banded butterfly (|out - in| <= ~101) plus a dense first row
(output 0 reads all inputs) and dense first column (input 0 feeds all
outputs).  At 128-block granularity W^T is block-tridiagonal (46 blocks)
plus the global row/col, so the kernel only does ~1/5 of the dense work.

Design (measured 73us on 8 cores, rel err 3.2e-4 vs the 2e-2 gate):

  - data-parallel over tokens: 8 shards of 2048 tokens, one per NeuronCore
  - fp16 single-pass everywhere: same PE rate as bf16 (1 col/cycle) with
    8 extra mantissa bits; fp8/DoubleRow variants fail the gate or cost
    as much as fp16 once enough correction terms are added
  - per output block bo (128 outputs), contract only over input blocks
    {bo-1, bo, bo+1}; x^T and the packed W^T band live resident in SBUF
  - PSUM groups are 1024 cols (2 banks): prewrite/evac engine ops carry
    ~350ns fixed overhead each, so double-width ops halve that tax
  - the dense i=0 input row (a rank-1 update) plus the bias are fused
    into one per-group PSUM pre-write on the ACT engine:
    ps = Identity(x0_bcast * w_in0[bo] + bias[bo]); the band matmuls
    accumulate on top with start=False.  This removes 56 PE matmuls.
    Every ps bank gets a start=True zero matmul at program start: the
    first start=False accumulation on a virgin bank does not reliably
    include engine-prewritten content.
  - x0_bcast comes from a GPSIMD partition_broadcast of row 0 of the
    first x load (no extra DMA); evacuation (psum -> fp16 staging) runs
    on the Vector engine; PE/ACT/DVE all land ~45us busy, balanced
  - the dense o=0 output column is 14 single-output (M=1) matmuls per
    token-slab, packed 4-wide into the PE array via tile_position column
    groups; the 4 partial rows are stored and folded into y[:, 0] on the
    host (unsharding the K-parallel reduction)
  - y is staged fp16 and stored fp16 ([out, tok], transposed) halving
    store traffic; the host un-transposes and upcasts
  - every dma_start costs ~600ns of sequencer descriptor-gen time, so
    loads are BATCHED into ~10 triggers each for x (partition-major
    layout makes any block range one contiguous-row transfer) and W
  - stores ride the sync queue, issued after all loads: the per-queue
    FIFO gives loads absolute priority for HBM bandwidth, and store
    triggers on the idle sync engine never block the ACT pre-writes
  - group order: bo=1 first (cheap, needs no x0 broadcast), 15 down to
    3, then bo=0 (needs all x blocks for the o=0 column), ending on the
    cheap bo=2

BGL_MODE env selects the dtype experiment (default fp16):
  fp16   - single fp16 pass (default)
  bf16   - single bf16 pass (for comparison)
"""

import os

import numpy as np
import ml_dtypes

import concourse.bass as bass  # noqa: F401  (bass types via bacc)
import concourse.mybir as mybir
import concourse.tile as tile
from concourse import bacc
from concourse.bass_utils import run_bass_kernel_spmd


def _ensure_axon_hooks():
    """run_bass_kernel_spmd(trace=True) imports antenv.axon_hooks, which some
    images lack. Register the real libaxon-backed hook if available, else a
    no-op, so a BASS_TRACE=1 environment profiles instead of crashing."""
    import sys
    import types

    try:
        import antenv.axon_hooks  # noqa: F401
        return
    except ImportError:
        pass
    hook = None
    try:
        from trn_agent_boot.trn_boot import _ntff_profile_via_ctypes

        hook = _ntff_profile_via_ctypes("/opt/axon/libaxon_pjrt.so")
    except Exception:
        hook = None
    mod = types.ModuleType("antenv.axon_hooks")
    mod.get_axon_ntff_profile_hook = lambda: hook
    sys.modules["antenv.axon_hooks"] = mod


_ensure_axon_hooks()

MODE = os.environ.get("BGL_MODE", "fp16")

N_CORES = 8
TOK = 16384
F = 2048
P = 128
NB = F // P            # 16 feature blocks
NFREE = 512            # psum free dim (one bank of fp32)

F32 = mybir.dt.float32
BF16 = mybir.dt.bfloat16
FP16 = mybir.dt.float16

# most recent run's results (exec_time_ns etc.) for test harnesses
LAST_RESULTS = None


def _kset(bo):
    """Band input blocks contracted for output block bo (tridiagonal)."""
    return [bi for bi in (bo - 1, bo, bo + 1) if 0 <= bi < NB]


# Start at bo=1: the only cheap group that does not read the x0
# broadcast (its band carries the global row), so the PE can start while
# the GPSIMD broadcast of x row 0 (sourced from the first x load) still
# runs. Then walk 15 down to 3 (each adds one new x block), bo=0 (needs
# every x block for the o=0 column) second-to-last, and end on bo=2
# (nothing new to load, 3 matmuls) for a cheap tail.
BO_ORDER = [1] + list(range(15, 2, -1)) + [0, 2]


def _load_kset(bo):
    """x blocks whose tiles bo's group consumes (bo=0 also feeds the
    column-packed o=0 global reduction over every block)."""
    return list(range(NB)) if bo == 0 else _kset(bo)


def _wblocks():
    """(bo, bi) pairs needing a W^T block, in device compute order (so the
    packed slab can be streamed in exactly the order it is consumed)."""
    return [(bo, bi) for bo in BO_ORDER for bi in _kset(bo)]


_NC_CACHE = {}


def _build_nc(mode, tok_sh):
    """Build + compile the per-core Bass module (SPMD: same NEFF, 8 cores)."""
    if (mode, tok_sh) in _NC_CACHE:
        return _NC_CACHE[(mode, tok_sh)]
    wdt = {"fp16": FP16, "bf16": BF16}[mode]
    ns_count = tok_sh // NFREE
    blocks = _wblocks()
    bidx = {pair: i for i, pair in enumerate(blocks)}

    nc = bacc.Bacc("TRN2", target_bir_lowering=False, debug=False)

    # W^T blocks host-packed dense: column slab j holds block (bo,bi)=blocks[j]
    w_dram = nc.dram_tensor("w", [P, len(blocks) * P], wdt, kind="ExternalInput")
    # x^T partition-major: row p holds [block0 | block1 | ...] so any
    # column-range load is one DMA trigger with 4-32KB contiguous rows
    x_dram = nc.dram_tensor("x", [P, NB * tok_sh], wdt, kind="ExternalInput")
    # bias and the dense-input W^T row, merged into one load:
    # [:, :NB] = bias ([p, bo] -> bias[bo*128+p]); [:, NB:] = gwrow
    # ([p, bo] -> W^T[0, bo*128+p], cols 0,1 zeroed on the host since
    # blocks 0/1 carry the global row via the band)
    bg_dram = nc.dram_tensor("bg_pf", [P, 2 * NB], F32, kind="ExternalInput")
    # o=0 output column of W^T, blocked: column bi = W^T[bi*128:(bi+1)*128, 0]
    w0_dram = nc.dram_tensor("w0col", [P, NB], wdt, kind="ExternalInput")
    yt_dram = nc.dram_tensor("yt", [F, tok_sh], wdt, kind="ExternalOutput")
    # o=0 column partial sums (one row per tile_position column group);
    # the host folds these into y[:, 0] (unsharding the K-parallel split)
    psg_dram = nc.dram_tensor("psg", [4, tok_sh], F32, kind="ExternalOutput")

    with tile.TileContext(nc) as tc:
        with (
            tc.tile_pool(name="wpool", bufs=1) as wpool,
            tc.tile_pool(name="xpool", bufs=1) as xpool,
            tc.tile_pool(name="gpool", bufs=1) as gpool,
            tc.tile_pool(name="pspool", bufs=8, space="PSUM") as pspool,
            tc.tile_pool(name="opool", bufs=2) as opool,
        ):
            # the merged bias/gwrow scalars are the FIRST sync-ring trigger
            # (one 16KB dma): the first pre-write needs them ~8us in, and
            # the ACT ring would serialize them behind its table load
            x0b_sb = gpool.tile([P, tok_sh], wdt, tag="x0b")
            bg_sb = gpool.tile([P, 2 * NB], F32, tag="bg")
            nc.sync.dma_start(bg_sb[:], bg_dram[:, :])
            w0_sb = gpool.tile([P, NB], wdt, tag="w0col")
            nc.scalar.dma_start(w0_sb[:], w0_dram[:, :])

            # x^T fully resident in one [128, NB*tok_sh] tile; block bi
            # occupies columns [bi*tok_sh, (bi+1)*tok_sh)
            xall = xpool.tile([P, NB * tok_sh], wdt, tag="xall", name="xall")
            # resident packed W^T slab; block j = [:, j*128:(j+1)*128]
            wsb = wpool.tile([P, len(blocks) * P], wdt, tag="w", name="w")

            # Issue loads in first-use order, but BATCHED: every dma_start
            # costs ~600ns of sequencer descriptor-gen time, so 33 per-tile
            # triggers would take ~20us just to issue. Singles up front for
            # a fast PE start, then progressively larger merged ranges.
            # x blocks in first-use order: 14,15,13,12,...,2,1,0 (descending
            # after the first two), so merged ranges stay contiguous.
            xbatches = [[0], [1, 2], [14, 15], [13], [12], [11, 10], [9, 8],
                        [7, 6], [5, 4], [3]]
            # W-chunk batches (j ranges follow BO_ORDER, so ranges merge)
            wbatches = [[1], [15], [14], [13], [12, 11], [10, 9], [8, 7],
                        [6, 5], [4, 3], [0, 2]]
            wseq = []
            for bos in wbatches:
                js = [bidx[(bo, bi)] for bo in bos for bi in _kset(bo)]
                wseq.append((min(js), max(js) + 1))
            # interleave: w batch, then x batch, in consumption order
            for k in range(max(len(xbatches), len(wseq))):
                if k < len(wseq):
                    jlo, jhi = wseq[k]
                    nc.sync.dma_start(
                        wsb[:, jlo * P : jhi * P], w_dram[:, jlo * P : jhi * P]
                    )
                if k < len(xbatches):
                    bis = xbatches[k]
                    lo, hi = min(bis), max(bis) + 1
                    nc.sync.dma_start(
                        xall[:, lo * tok_sh : hi * tok_sh],
                        x_dram[:, lo * tok_sh : hi * tok_sh],
                    )
                if k == 0:
                    # broadcast x row 0 (= partition 0 of the first x load)
                    # across partitions for the global-input pre-writes; on
                    # the idle GPSIMD, split in halves so the second group's
                    # pre-write (which reads the first half) unblocks ~1.6us
                    # sooner
                    nc.gpsimd.partition_broadcast(
                        x0b_sb[:, 0 : tok_sh // 2],
                        xall[0:1, 0 : tok_sh // 2],
                    )
                    nc.gpsimd.partition_broadcast(
                        x0b_sb[:, tok_sh // 2 : tok_sh],
                        xall[0:1, tok_sh // 2 : tok_sh],
                    )

            nw = 2 * NFREE  # 1024-col groups: 2 psum banks, halves the
            nw_count = tok_sh // nw  # fixed per-op overhead of prewrite/evac

            # Prime every ps bank with a start=True zero matmul before the
            # real groups: the first start=False accumulation on a virgin
            # bank does not reliably include engine-prewritten content
            # (observed as garbage in exactly the first use of each bank).
            # Zero operands come from memsets, so priming runs during the
            # DMA ramp and costs nothing on the critical path.
            zcol = gpool.tile([1, P], wdt, tag="zcol", name="zcol")
            nc.gpsimd.memset(zcol[:], 0.0)
            zrow = gpool.tile([1, NFREE], wdt, tag="zrow", name="zrow")
            nc.gpsimd.memset(zrow[:], 0.0)
            for _ in range(3):
                pp = pspool.tile([P, nw], F32, tag="ps", bufs=3)
                for h in range(2):
                    nc.tensor.matmul(
                        pp[:, h * NFREE : (h + 1) * NFREE],
                        zcol[:],
                        zrow[:],
                        start=True,
                        stop=True,
                    )
            # (PE warm-up matmuls were tried here and measured neutral: the
            # DVFS step to full clock happens at a ~fixed time after kernel
            # start regardless of PE activity pattern.)

            for gi, bo in enumerate(BO_ORDER):
                ostage = opool.tile([P, tok_sh], wdt, tag="o", bufs=16)
                for nsw in range(nw_count):
                    wsl = slice(nsw * nw, (nsw + 1) * nw)
                    ps2 = pspool.tile([P, nw], F32, tag="ps", bufs=3)

                    # For groups with a global-input term (bo >= 2): fused
                    # bias + dense-input rank-1 pre-write on the ACT engine,
                    # ps = Identity(x0b * w_in0[bo] + bias[bo]), matmuls
                    # accumulate with start=False. The bias-only groups
                    # (bo 0/1: gwrow is 0, their band carries the global
                    # row) skip the pre-write entirely and start=True, so
                    # the FIRST group (bo=1) is unblocked the moment its
                    # weights and x arrive; their bias rides the evac.
                    has_global = bo >= 2
                    if has_global:
                        nc.scalar.activation(
                            ps2[:],
                            x0b_sb[:, wsl],
                            mybir.ActivationFunctionType.Identity,
                            bias=bg_sb[:, bo : bo + 1],
                            scale=bg_sb[:, NB + bo : NB + bo + 1],
                        )

                    ks = _kset(bo)
                    for h in range(2):
                        tsl = slice((2 * nsw + h) * NFREE, (2 * nsw + h + 1) * NFREE)
                        for i, bi in enumerate(ks):
                            j = bidx[(bo, bi)]
                            nc.tensor.matmul(
                                ps2[:, h * NFREE : (h + 1) * NFREE],
                                wsb[:, j * P : (j + 1) * P],
                                xall[:, bi * tok_sh + tsl.start : bi * tok_sh + tsl.stop],
                                start=(not has_global and i == 0),
                                stop=(i == len(ks) - 1),
                                skip_group_check=True,
                            )

                    # evacuate psum -> fp16 staging on DVE (the ACT engine is
                    # busy with the pre-writes); the final group's evac is
                    # split across ACT+DVE so the last store starts ~0.6us
                    # earlier (ACT is idle by then)
                    if not has_global:
                        # bias-only groups: bias is applied here instead of
                        # a pre-write (ACT is idle at both points these run)
                        nc.scalar.activation(
                            ostage[:, wsl],
                            ps2[:],
                            mybir.ActivationFunctionType.Identity,
                            bias=bg_sb[:, bo : bo + 1],
                        )
                    elif gi == len(BO_ORDER) - 1:
                        nc.scalar.activation(
                            ostage[:, wsl.start : wsl.start + NFREE],
                            ps2[:, 0:NFREE],
                            mybir.ActivationFunctionType.Copy,
                        )
                        nc.vector.tensor_copy(
                            ostage[:, wsl.start + NFREE : wsl.stop],
                            ps2[:, NFREE : nw],
                        )
                    else:
                        nc.vector.tensor_copy(ostage[:, wsl], ps2[:])

                if bo == 0:
                    # o=0 global column: every block bi>=2 contributes a
                    # single-output (M=1) matmul. Pack them 4-wide into the
                    # PE array via tile_position column groups so four
                    # stream concurrently; the 4 partial rows are stored and
                    # folded into y[:, 0] on the host (unsharding the
                    # K-parallel reduction).
                    psg_stage = gpool.tile(
                        [P, tok_sh], F32, tag="psg_stage", name="psg_stage"
                    )
                    units = list(range(2, NB))
                    ngrp = 4
                    per_grp = [[] for _ in range(ngrp)]
                    for idx, u in enumerate(units):
                        per_grp[idx % ngrp].append(u)
                    order = []
                    for slot in range(max(len(g) for g in per_grp)):
                        for jg in range(ngrp):
                            if slot < len(per_grp[jg]):
                                order.append((jg, slot, per_grp[jg][slot]))
                    for ns in range(ns_count):
                        tsl = slice(ns * NFREE, (ns + 1) * NFREE)
                        psg = pspool.tile(
                            [P, NFREE], F32, tag="psg", bufs=2, name="psg"
                        )
                        for jg, slot, bi in order:
                            nc.tensor.matmul(
                                psg[32 * jg : 32 * jg + 1, :],
                                w0_sb[:, bi : bi + 1],
                                xall[:, bi * tok_sh + tsl.start : bi * tok_sh + tsl.stop],
                                start=(slot == 0),
                                stop=(slot == len(per_grp[jg]) - 1),
                                tile_position=(0, 32 * jg),
                            )
                        # DMA cannot read PSUM: stage partitions 0..96 to
                        # SBUF in one DVE copy (cost is column-driven); the
                        # batched store happens after the slab loop
                        nc.vector.tensor_copy(
                            psg_stage[0:97, tsl], psg[0:97, :]
                        )

                if bo == 0:
                    for jg in range(4):
                        nc.sync.dma_start(
                            psg_dram[jg, :],
                            psg_stage[32 * jg : 32 * jg + 1, :],
                        )

                # stores ride the SYNC queue, issued after every load: the
                # per-queue FIFO then gives loads absolute priority over
                # stores for HBM bandwidth (no mid-run contention), and the
                # idle sync engine's store triggers never block the ACT
                # queue's pre-writes. ostage is buffered per-group (bufs=16)
                # so held-back stores don't stall evacuation.
                if gi == len(BO_ORDER) - 1:
                    for nsw in range(nw_count):
                        wsl = slice(nsw * nw, (nsw + 1) * nw)
                        nc.sync.dma_start(
                            yt_dram[bo * P : (bo + 1) * P, wsl], ostage[:, wsl]
                        )
                else:
                    nc.sync.dma_start(
                        yt_dram[bo * P : (bo + 1) * P, :], ostage[:]
                    )

    nc.compile()
    _NC_CACHE[(mode, tok_sh)] = nc
    return nc


def _prep_inputs(x, mask, weight, bias, mode, tok_sh):
    """Host-side layout prep -> per-core input maps."""
    npdt = {"fp16": np.float16, "bf16": ml_dtypes.bfloat16}[mode]
    n_sh = x.shape[0] // tok_sh

    w = mask.astype(np.float32) * weight.astype(np.float32)
    wtr = np.ascontiguousarray(w.T)  # [in, out]

    # pack the needed W^T blocks into a dense [128, nblocks*128] slab
    blocks = _wblocks()
    packed = np.empty((P, len(blocks) * P), dtype=np.float32)
    for j, (bo, bi) in enumerate(blocks):
        packed[:, j * P : (j + 1) * P] = wtr[
            bi * P : (bi + 1) * P, bo * P : (bo + 1) * P
        ]
    w_pk = packed.astype(npdt)

    # o=0 output column of W^T, blocked [128, NB]
    w0col = np.ascontiguousarray(wtr[:, 0].reshape(NB, P).T).astype(npdt)

    bias_pf = bias.astype(np.float32).reshape(NB, P).T

    # dense-input row of W^T, blocked [128, NB]; zero the columns whose
    # band blocks already carry the global row (input block 0 in bo=0,1)
    gwrow = wtr[0, :].astype(np.float32).reshape(NB, P).T.copy()
    gwrow[:, 0] = 0.0
    gwrow[:, 1] = 0.0
    bg_pf = np.ascontiguousarray(np.concatenate([bias_pf, gwrow], axis=1))

    # per-core transposed x shards, partition-major: [core, 128, NB*tok]
    xs = x.reshape(n_sh, tok_sh, F).transpose(0, 2, 1)
    x_h = np.ascontiguousarray(xs).astype(npdt).reshape(n_sh, NB, P, tok_sh)
    x_pm = np.ascontiguousarray(x_h.transpose(0, 2, 1, 3)).reshape(
        n_sh, P, NB * tok_sh
    )

    in_maps = []
    for c in range(n_sh):
        in_maps.append(
            {
                "bg_pf": bg_pf,
                "w": w_pk,
                "w0col": w0col,
                "x": x_pm[c],
            }
        )
    return in_maps


def kernel(x, mask, weight, bias):
    global LAST_RESULTS
    x = np.asarray(x)
    tok, f = x.shape
    assert (tok, f) == (TOK, F), (tok, f)
    tok_sh = tok // N_CORES

    nc = _build_nc(MODE, tok_sh)
    in_maps = _prep_inputs(
        np.asarray(x), np.asarray(mask), np.asarray(weight), np.asarray(bias),
        MODE, tok_sh,
    )
    res = run_bass_kernel_spmd(nc, in_maps, list(range(N_CORES)))
    LAST_RESULTS = res

    y = np.empty((tok, F), dtype=np.float32)
    for c in range(N_CORES):
        sl = slice(c * tok_sh, (c + 1) * tok_sh)
        y[sl, :] = res.results[c]["yt"].T.astype(np.float32)
        # unshard the K-parallel o=0 column reduction: fold the 4
        # column-group partial rows into y[:, 0]
        y[sl, 0] += res.results[c]["psg"].astype(np.float32).sum(axis=0)
    return y
